# revision 20
# baseline (speedup 1.0000x reference)
"""Trainium2 Bass kernel for Baichuan attention (B=2, S=2048, H=4096, 32 heads).

Sharding: 8 cores = 2 (batch) x 4 (head groups of 8 heads), tensor-parallel
mirror of ColumnParallelLinear/RowParallelLinear. Each core computes, for its
batch b and head group g:
    qkT   = (w_pack q,k slice) @ x_b.T        [2048 qkdims, 2048 seq]
            in fp8e4 DoubleRow (x,w scaled by 512 each; descale folded into
            the cos/sin rope tables), rope via a signed half-swap matmul
    v     = x_b @ (w_pack v slice).T          [2048 seq, 1024]  (bf16),
            written straight into a persistent SBUF tile (no DRAM roundtrip)
    per head: scoresT tiles -> exp -> causal mask
              l = ones @ eP (softmax denominators; full tiles pre-summed in
              quads on the DVE so the PE does 1/4 of the ones-matmuls)
              out_hT = sum v eP
    partial = attn_out @ w_o[:, cols].T       [2048, 4096]             (f32)
Host sums the 4 TP partials per batch (row-parallel all-reduce done on host).

Self-contained: hardcodes all shapes; only needs concourse + numpy + ml_dtypes.
"""
import math
from contextlib import ExitStack

import numpy as np
import ml_dtypes

import concourse.bass as bass
import concourse.mybir as mybir
import concourse.tile as tile
from concourse import bacc
from concourse.bass_utils import run_bass_kernel_spmd

bf16 = ml_dtypes.bfloat16
f8 = ml_dtypes.float8_e4m3
FP32 = mybir.dt.float32
BF16 = mybir.dt.bfloat16
F8E4 = mybir.dt.float8e4
DR = mybir.MatmulPerfMode.DoubleRow

B, S, H = 2, 2048, 4096
NH_TOT, HD = 32, 128
NHL = 8                # heads per core
KT = H // 128          # 32 contraction tiles for the projections
VD = NHL * HD          # 1024 local v dims
SCALE = 1.0 / math.sqrt(HD)
ROPE_BASE = 10000.0
SX = 512.0             # fp8 input scale for x
SW = 512.0             # fp8 input scale for w_pack qk rows

_NC_CACHE = {}


def build_nc():
    nc = bacc.Bacc()
    x = nc.declare_dram_parameter("x", [KT, 128, S], BF16, isOutput=False)
    x8 = nc.declare_dram_parameter("x8", [KT, 128, S], F8E4, isOutput=False)
    wqk = nc.declare_dram_parameter("wqk", [16, KT, 128, 128], F8E4, isOutput=False)
    wv = nc.declare_dram_parameter("wv", [4, 16, 128, 512], BF16, isOutput=False)
    wo = nc.declare_dram_parameter("wo", [8, 8, 128, 512], BF16, isOutput=False)
    cos = nc.declare_dram_parameter("cos", [128, S], BF16, isOutput=False)
    sin = nc.declare_dram_parameter("sin", [128, S], BF16, isOutput=False)
    msk = nc.declare_dram_parameter("msk", [128, 128], BF16, isOutput=False)
    ones = nc.declare_dram_parameter("ones", [128, 128], BF16, isOutput=False)
    out = nc.declare_dram_parameter("out", [S, H], FP32, isOutput=True)

    EXP = mybir.ActivationFunctionType.Exp

    with tile.TileContext(nc) as tc, ExitStack() as g:
        glob = g.enter_context(tc.tile_pool(name="glob", bufs=1))

        qkrot = [glob.tile([128, S], BF16, tag=f"qkrot{t}", name=f"qkrot{t}")
                 for t in range(16)]
        v_all = glob.tile([128, 16, VD], BF16, tag="vall", name="vall")
        ones_sb = glob.tile([128, 128], BF16, tag="ones", name="ones_sb")
        nc.sync.dma_start(out=ones_sb[:], in_=ones[:])

        # ---------- phase 1a: qk projection (fp8 DoubleRow) + rope ---------
        if True:
            with ExitStack() as s1:
                c1 = s1.enter_context(tc.tile_pool(name="c1", bufs=1))
                xpool = s1.enter_context(tc.tile_pool(name="xp", bufs=1))
                wpool = s1.enter_context(tc.tile_pool(name="wp", bufs=2))
                evict = s1.enter_context(tc.tile_pool(name="ev", bufs=3))
                swpool = s1.enter_context(tc.tile_pool(name="sw", bufs=2))
                rope = s1.enter_context(tc.tile_pool(name="rope", bufs=1))
                pp = s1.enter_context(tc.tile_pool(name="pp", bufs=2, space="PSUM"))

                cos_sb = c1.tile([128, S], BF16, tag="cos", name="cos_sb")
                sin_sb = c1.tile([128, S], BF16, tag="sin", name="sin_sb")
                nc.sync.dma_start(out=cos_sb[:], in_=cos[:])
                nc.sync.dma_start(out=sin_sb[:], in_=sin[:])

                for hf in range(2):
                    # two separate 512-col tiles: the next half's load of one
                    # slab only waits on THAT slab's readers (whole-tile WAR
                    # otherwise).  x loads ride the Scalar HWDGE queue in
                    # 4-ktile chunks so they never head-of-line-block the wqp
                    # panel stream (Sync queue) and the first matmul starts
                    # after ~256KB, not 4MB.
                    xh_lo = xpool.tile([128, KT, 512], F8E4, tag="xhlo", name=f"xhlo{hf}")
                    xh_hi = xpool.tile([128, KT, 512], F8E4, tag="xhhi", name=f"xhhi{hf}")
                    for slab, xq in ((xh_lo, 0), (xh_hi, 1)):
                        for kh in range(8):
                            nc.scalar.dma_start(
                                out=slab[:, kh * 4:(kh + 1) * 4, :],
                                in_=x8[kh * 4:(kh + 1) * 4,
                                       :, hf * 1024 + xq * 512: hf * 1024 + (xq + 1) * 512]
                                .rearrange("k p s -> p k s"),
                            )

                    # rope for block n is emitted after block n+1's projection
                    # matmuls; the half-swap rides the DMA engines (two
                    # partition-half copies, sign folded into the sin table)
                    # so the PE does no swap matmul at all.
                    pending = None

                    def flush_rope(pend, hf=hf):
                        qkraw, sw, mt_p, sb_p = pend
                        t1 = rope.tile([128, 512], BF16, tag="t1", name=f"t1_{hf}{mt_p}{sb_p}")
                        t2 = rope.tile([128, 512], BF16, tag="t2", name=f"t2_{hf}{mt_p}{sb_p}")
                        nc.vector.tensor_mul(t1[:], qkraw[:], cos_sb[:, sb_p:sb_p + 512])
                        nc.vector.tensor_mul(t2[:], sw[:], sin_sb[:, sb_p:sb_p + 512])
                        nc.vector.tensor_add(qkrot[mt_p][:, sb_p:sb_p + 512], t1[:], t2[:])

                    # nb2-outer so each 512-col slab of xh has its last reader
                    # at the end of one sub-phase: the next half's x DMA for
                    # that slab overlaps the other slab's compute
                    for nb2 in range(2):
                        for mt in range(16):
                            wqp = wpool.tile([128, KT, 128], F8E4, tag="wqp",
                                             name=f"wqp{hf}{nb2}{mt}")
                            # two DMAs so the panel loads stay ahead of the PE
                            nc.sync.dma_start(out=wqp[:, 0:16, :],
                                              in_=wqk[mt, 0:16].rearrange("k p c -> p k c"))
                            nc.sync.dma_start(out=wqp[:, 16:32, :],
                                              in_=wqk[mt, 16:32].rearrange("k p c -> p k c"))
                            sb = hf * 1024 + nb2 * 512
                            pqk = pp.tile([128, 512], FP32, tag="pqk", name=f"pqk{hf}{mt}{nb2}")
                            xslab = xh_lo if nb2 == 0 else xh_hi
                            for k2 in range(0, KT, 2):
                                nc.tensor.matmul(
                                    pqk[:],
                                    lhsT=wqp[:, k2:k2 + 2, :],
                                    rhs=xslab[:, k2:k2 + 2, :],
                                    start=(k2 == 0),
                                    stop=(k2 == KT - 2),
                                    perf_mode=DR,
                                )
                            qkraw = evict.tile([128, 512], BF16, tag="qkraw",
                                               name=f"qkraw{hf}{mt}{nb2}")
                            nc.scalar.copy(qkraw[:], pqk[:])
                            sw = swpool.tile([128, 512], BF16, tag="sw",
                                             name=f"sw{hf}{mt}{nb2}")
                            nc.scalar.dma_start(out=sw[0:64, :], in_=qkraw[64:128, :])
                            nc.scalar.dma_start(out=sw[64:128, :], in_=qkraw[0:64, :])
                            if pending is not None:
                                flush_rope(pending)
                            pending = (qkraw, sw, mt, sb)
                    if pending is not None:
                        flush_rope(pending)
                        pending = None

            # ------ phase 1b: v projection (bf16), straight into SBUF ------
            with ExitStack() as s1b:
                xpoolv = s1b.enter_context(tc.tile_pool(name="xpv", bufs=1))
                wpool = s1b.enter_context(tc.tile_pool(name="wpb", bufs=2))
                pp = s1b.enter_context(tc.tile_pool(name="ppb", bufs=2, space="PSUM"))

                def load_vx(hf, xq):
                    # 8 chunks alternating between the two HWDGE queues: the
                    # slab fills at ~2x one queue's rate at phase entry
                    slab = xpoolv.tile([128, KT, 512], BF16, tag=f"vxh{xq}",
                                       name=f"vxh{hf}{xq}")
                    for kh in range(8):
                        eng = nc.scalar if kh % 2 == 0 else nc.sync
                        eng.dma_start(
                            out=slab[:, kh * 4:(kh + 1) * 4, :],
                            in_=x[kh * 4:(kh + 1) * 4,
                                  :, hf * 1024 + xq * 512: hf * 1024 + (xq + 1) * 512]
                            .rearrange("k p s -> p k s"),
                        )
                    return slab

                for hf in range(2):
                    xh_lo = load_vx(hf, 0)
                    xh_hi = load_vx(hf, 1)

                    def xh_chunk(k, c0, w, xh_lo=xh_lo, xh_hi=xh_hi):
                        # columns [c0, c0+w) of this half's x, no slab cross
                        slab = xh_lo if c0 < 512 else xh_hi
                        cc = c0 % 512
                        assert cc + w <= 512
                        return slab[:, k, cc:cc + w]

                    # v projection: v[seq, vdim] natural layout; N=512 panels
                    # split into two half-K chunks so a bufs=2 pool
                    # double-buffers 1MB DMAs
                    for nb in range(2):
                        panels = []
                        for kh in range(2):
                            wvp = wpool.tile([128, 16, 512], BF16, tag="wvp",
                                             name=f"wvp{hf}{nb}{kh}")
                            nc.sync.dma_start(out=wvp[:, 0:8, :],
                                              in_=wv[2 * nb + kh, 0:8].rearrange("k p c -> p k c"))
                            nc.sync.dma_start(out=wvp[:, 8:16, :],
                                              in_=wv[2 * nb + kh, 8:16].rearrange("k p c -> p k c"))
                            panels.append(wvp)
                        for mt in range(8):
                            pv = pp.tile([128, 512], FP32, tag="pv", name=f"pv{hf}{nb}{mt}")
                            for k in range(KT):
                                nc.tensor.matmul(
                                    pv[:],
                                    lhsT=xh_chunk(k, mt * 128, 128),
                                    rhs=panels[k // 16][:, k % 16, :],
                                    start=(k == 0),
                                    stop=(k == KT - 1),
                                )
                            st = hf * 8 + mt
                            # DVE eviction keeps the ACT queue free so the next
                            # half's x-slab DMA triggers fire immediately
                            nc.vector.tensor_copy(
                                v_all[:, st, nb * 512:(nb + 1) * 512], pv[:])

        # ---------- phases 2+3: attention, then output projection ----------
        with ExitStack() as s2:
            c2 = s2.enter_context(tc.tile_pool(name="c2", bufs=1))
            apool = s2.enter_context(tc.tile_pool(name="ap", bufs=1))
            eppool = s2.enter_context(tc.tile_pool(name="ep", bufs=6))
            sqpool = s2.enter_context(tc.tile_pool(name="sq", bufs=3))
            s2p = s2.enter_context(ExitStack())
            att_ps = s2p.enter_context(tc.tile_pool(name="attps", bufs=3, space="PSUM"))
            av_ps = s2p.enter_context(tc.tile_pool(name="avps", bufs=2, space="PSUM"))
            l_ps = s2p.enter_context(tc.tile_pool(name="lps", bufs=1, space="PSUM"))
            bc_ps = s2p.enter_context(tc.tile_pool(name="bcps", bufs=1, space="PSUM"))

            msk_sb = c2.tile([128, 128], BF16, tag="msk", name="msk_sb")
            nc.sync.dma_start(out=msk_sb[:], in_=msk[:])

            attnT = [apool.tile([128, S], BF16, tag=f"attnT{t}", name=f"attnT{t}")
                     for t in range(8)]

            # one l bank pair reused across all j; garbage rows only ever feed
            # unused reciprocal lanes
            lA = l_ps.tile([128, 512], FP32, tag="lA", name="lA")
            lB = l_ps.tile([128, 512], FP32, tag="lB", name="lB")
            nc.vector.memset(lA[:], 1.0)
            nc.vector.memset(lB[:], 1.0)

            lsaves = []
            for j in range(4):
                ni = 4 * j + 4
                for h in range(8):
                    lbank = lA if h < 4 else lB
                    hp = (h % 4) * 32
                    pav = av_ps.tile([128, 512], FP32, tag="pav", name=f"pav{j}{h}")

                    eps = {}

                    def c_lo(i, j=j):
                        r = i - 4 * j
                        return 128 * r if r > 0 else 0

                    def emit_score(i, j=j, h=h):
                        # diagonal tiles only need columns >= 128*(i-4j); the
                        # strip [c0, c0+128) gets the triangular mask
                        c0 = c_lo(i)
                        psc = att_ps.tile([128, 512], FP32, tag="psc",
                                          name=f"psc{j}{h}{i}")
                        nc.tensor.matmul(
                            psc[:, c0:512],
                            lhsT=qkrot[8 + h][:, i * 128:(i + 1) * 128],
                            rhs=qkrot[h][:, j * 512 + c0:(j + 1) * 512],
                            start=True, stop=True,
                        )
                        ep = eppool.tile([128, 512], BF16, tag="ep",
                                         name=f"ep{j}{h}{i}")
                        nc.scalar.activation(ep[:, c0:512], psc[:, c0:512],
                                             EXP, scale=SCALE)
                        if i - 4 * j >= 0:
                            nc.vector.tensor_mul(ep[:, c0:c0 + 128],
                                                 ep[:, c0:c0 + 128], msk_sb[:])
                        eps[i] = ep

                    # l reduction plan: full tiles (i < 4j) are pre-summed in
                    # quads on the DVE (one ones-matmul per 4 tiles); the 4
                    # ragged diagonal tiles go straight to the PE.  The quad
                    # l-matmuls are deferred to group end so the PE (in-order)
                    # never waits on the DVE adds.
                    quad = []   # full-width ep tiles awaiting quad reduction
                    qqs = []    # reduced quad tiles awaiting their l-matmul

                    def flush_quad(j=j, h=h):
                        assert len(quad) == 4
                        nq = len(qqs)
                        q0 = sqpool.tile([128, 512], BF16, tag="q0",
                                         name=f"q0_{j}{h}{nq}")
                        q1 = sqpool.tile([128, 512], BF16, tag="q1",
                                         name=f"q1_{j}{h}{nq}")
                        qq = sqpool.tile([128, 512], BF16, tag="qq",
                                         name=f"qq_{j}{h}{nq}")
                        nc.vector.tensor_add(q0[:], quad[0][:], quad[1][:])
                        nc.vector.tensor_add(q1[:], quad[2][:], quad[3][:])
                        nc.vector.tensor_add(qq[:], q0[:], q1[:])
                        qqs.append(qq)
                        quad.clear()

                    # software-pipeline: scores run 3 tiles ahead of l/av so the
                    # exp+mask latency never stalls the PE
                    LOOKAHEAD = 3
                    for i in range(ni + LOOKAHEAD):
                        if i < ni:
                            emit_score(i)
                        ii = i - LOOKAHEAD
                        if ii < 0:
                            continue
                        ep = eps.pop(ii)
                        c0 = c_lo(ii)
                        if ii < 4 * j:
                            quad.append(ep)
                            if len(quad) == 4:
                                flush_quad()
                        else:
                            # ragged diagonal tile: direct l-matmul.  These run
                            # first in the group (quads are deferred), so the
                            # first one (full-width, c0=0) carries start=True.
                            nc.tensor.matmul(
                                lbank[hp:hp + 1, c0:512],
                                lhsT=ones_sb[:, 0:1],
                                rhs=ep[:, c0:512],
                                start=(ii == 4 * j),
                                stop=(j == 0 and ii == ni - 1),
                                tile_position=(0, hp),
                            )
                        nc.tensor.matmul(
                            pav[:, c0:512],
                            lhsT=v_all[:, ii, h * 128:(h + 1) * 128],
                            rhs=ep[:, c0:512],
                            start=(ii == 0), stop=(ii == ni - 1),
                        )
                    assert not quad
                    for qi, qq in enumerate(qqs):
                        nc.tensor.matmul(
                            lbank[hp:hp + 1, :],
                            lhsT=ones_sb[:, 0:1],
                            rhs=qq[:],
                            start=False, stop=(qi == len(qqs) - 1),
                            tile_position=(0, hp),
                        )
                    # DVE copy: keeps the ScalarE exp-only during attention (no
                    # activation-table thrash between Copy and Exp)
                    nc.vector.tensor_copy(attnT[h][:, j * 512:(j + 1) * 512], pav[:])
                    # evict each l bank right after its LAST writer (lA: h==3,
                    # lB: h==7) so the next j's first l-matmul never waits on a
                    # queued DVE copy (WAR on the bank)
                    if h == 3:
                        lsA = apool.tile([128, 512], FP32, tag=f"lsA{j}", name=f"lsA{j}")
                        nc.vector.tensor_copy(lsA[:], lA[:])
                    elif h == 7:
                        lsB = apool.tile([128, 512], FP32, tag=f"lsB{j}", name=f"lsB{j}")
                        nc.vector.tensor_copy(lsB[:], lB[:])
                # the slow reciprocals run after the j loop, off the PE
                # critical path
                lsaves.append((lsA, lsB))

            # bf16 reciprocals: lets the broadcast matmul run in bf16 (a single
            # pass) instead of the two-pass fp32 LOW/HIGH mode.  Chunked into
            # 128-col pieces: the iterative-divide op runs 8 cyc/elem, and a
            # full-width one parks 3.3us on the DVE queue right where the
            # attention loop's quick mask/quad ops need it (head-of-line PE
            # stalls measured at every reciprocal).
            linvs = []
            for j in range(4):
                linvA = apool.tile([128, 512], BF16, tag=f"linvA{j}", name=f"linvA{j}")
                linvB = apool.tile([128, 512], BF16, tag=f"linvB{j}", name=f"linvB{j}")
                with nc.allow_low_precision(reason="bf16 1/l: +1e-3 rel err, single-pass bcast matmul"):
                    for cc in range(0, 512, 128):
                        nc.vector.reciprocal(linvA[:, cc:cc + 128],
                                             lsaves[j][0][:, cc:cc + 128])
                        nc.vector.reciprocal(linvB[:, cc:cc + 128],
                                             lsaves[j][1][:, cc:cc + 128])
                linvs.append((linvA, linvB))

            # normalize attn_outT by 1/l (broadcast 1/l across partitions)
            for j in range(4):
                for h in range(8):
                    linv = linvs[j][0] if h < 4 else linvs[j][1]
                    hp = (h % 4) * 32
                    bc = bc_ps.tile([128, 512], FP32, tag="bc", name=f"bc{j}{h}")
                    nc.tensor.matmul(
                        bc[:],
                        lhsT=ones_sb[hp:hp + 1, :],
                        rhs=linv[hp:hp + 1, :],
                        start=True, stop=True,
                        tile_position=(hp, 0),
                    )
                    nc.vector.tensor_mul(
                        attnT[h][:, j * 512:(j + 1) * 512],
                        attnT[h][:, j * 512:(j + 1) * 512],
                        bc[:],
                    )

            # ---------- phase 3: output projection --------------------------
            s2p.close()
            with ExitStack() as s3:
                wpool3 = s3.enter_context(tc.tile_pool(name="wp3", bufs=2))
                ev3 = s3.enter_context(tc.tile_pool(name="ev3", bufs=3))
                po_ps = s3.enter_context(tc.tile_pool(name="pops", bufs=2, space="PSUM"))
                for nb in range(8):
                    # wop rides the Scalar HWDGE queue so it is never stuck
                    # behind the 4MB/nb of output-tile writes on the Sync queue
                    wop = wpool3.tile([128, 8, 512], BF16, tag="wop", name=f"wop{nb}")
                    nc.scalar.dma_start(out=wop[:, 0:4, :],
                                        in_=wo[nb, 0:4].rearrange("k p c -> p k c"))
                    nc.scalar.dma_start(out=wop[:, 4:8, :],
                                        in_=wo[nb, 4:8].rearrange("k p c -> p k c"))
                    for mt in range(16):
                        po = po_ps.tile([128, 512], FP32, tag="po", name=f"po{nb}{mt}")
                        for k in range(8):
                            nc.tensor.matmul(
                                po[:],
                                lhsT=attnT[k][:, mt * 128:(mt + 1) * 128],
                                rhs=wop[:, k, :],
                                start=(k == 0), stop=(k == 7),
                            )
                        osb = ev3.tile([128, 512], FP32, tag="osb", name=f"osb{nb}{mt}")
                        nc.scalar.copy(osb[:], po[:])
                        nc.sync.dma_start(
                            out=out[mt * 128:(mt + 1) * 128, nb * 512:(nb + 1) * 512],
                            in_=osb[:],
                        )

    nc.finalize()
    return nc


def _rope_tables(pos_row):
    """cos/sin tables [128, S]: row p uses inv_freq[p % 64].  The 1/(SX*SW)
    fp8 descale for q,k is folded in, and sin carries the half-swap sign
    (rows 0..63 multiply the swapped-in upper half: factor -sin)."""
    inv = 1.0 / (ROPE_BASE ** (np.arange(0, HD, 2, dtype=np.float32) / HD))  # [64]
    inv128 = np.concatenate([inv, inv]).astype(np.float32)                   # [128]
    ang = inv128[:, None] * pos_row[None, :].astype(np.float32)              # [128, S]
    ds = 1.0 / (SX * SW)
    sgn = np.concatenate([-np.ones(64, np.float32), np.ones(64, np.float32)])
    return (np.cos(ang) * ds).astype(bf16), (np.sin(ang) * ds * sgn[:, None]).astype(bf16)


def _consts():
    # triangular tile mask: msk[p, c] = 1 iff c >= p
    msk = np.triu(np.ones((128, 128), np.float32))
    ones = np.ones((128, 128), np.float32)
    return msk.astype(bf16), ones.astype(bf16)


def prep_in_maps(hidden_states, w_pack, w_o, positions):
    hidden_states = np.asarray(hidden_states, dtype=np.float32)
    w_pack = np.asarray(w_pack, dtype=np.float32)
    w_o = np.asarray(w_o, dtype=np.float32)
    positions = np.asarray(positions)

    msk, ones = _consts()
    in_maps = []
    for c in range(8):
        b, g = divmod(c, 4)
        xT = np.ascontiguousarray(hidden_states[b].T)                  # [H, S]
        x_np = xT.astype(bf16).reshape(KT, 128, S)
        x8_np = np.clip(xT * SX, -240, 240).astype(f8).reshape(KT, 128, S)
        qbase = g * 1024
        kbase = H + g * 1024
        vbase = 2 * H + g * 1024
        wqk_np = np.empty((16, KT, 128, 128), f8)
        for mt in range(16):
            base = qbase + 128 * mt if mt < 8 else kbase + 128 * (mt - 8)
            blk = w_pack[base:base + 128, :]                      # [128, H]
            wqk_np[mt] = np.clip(blk.T * SW, -240, 240).astype(f8).reshape(KT, 128, 128)
        wv_np = np.empty((4, 16, 128, 512), bf16)
        for nb in range(2):
            blk = w_pack[vbase + 512 * nb: vbase + 512 * (nb + 1), :]  # [512, H]
            arr = blk.T.astype(bf16).reshape(2, 16, 128, 512)          # [kh, kk, p, c]
            wv_np[2 * nb] = arr[0]
            wv_np[2 * nb + 1] = arr[1]
        woT = np.ascontiguousarray(w_o[:, g * 1024:(g + 1) * 1024].T)  # [1024, H]
        wo_np = np.ascontiguousarray(
            woT.reshape(8, 128, 8, 512).transpose(2, 0, 1, 3)
        ).astype(bf16)
        cos_np, sin_np = _rope_tables(positions[b])
        in_maps.append({
            "x": x_np, "x8": x8_np, "wqk": wqk_np, "wv": wv_np, "wo": wo_np,
            "cos": cos_np, "sin": sin_np,
            "msk": msk, "ones": ones,
        })
    return in_maps


def kernel(hidden_states, w_pack, w_o, positions, _run_kwargs=None):
    if "nc" not in _NC_CACHE:
        _NC_CACHE["nc"] = build_nc()
    nc = _NC_CACHE["nc"]
    in_maps = prep_in_maps(hidden_states, w_pack, w_o, positions)
    res = run_bass_kernel_spmd(nc, in_maps, core_ids=list(range(8)),
                               **(_run_kwargs or {}))
    _NC_CACHE["last_result"] = res
    out = np.zeros((B, S, H), np.float32)
    for c in range(8):
        b = c // 4
        out[b] += res.results[c]["out"]
    return out


# revision 28
# speedup vs baseline: 1.0098x; 1.0098x over previous
"""Trainium2 Bass kernel for Baichuan attention (B=2, S=2048, H=4096, 32 heads).

Sharding: 8 cores = 2 (batch) x 4 (head groups of 8 heads), tensor-parallel
mirror of ColumnParallelLinear/RowParallelLinear. Each core computes, for its
batch b and head group g:
    qkT   = (w_pack q,k slice) @ x_b.T        [2048 qkdims, 2048 seq]
            in fp8e4 DoubleRow (x,w scaled by 512 each; descale folded into
            the cos/sin rope tables), rope via a signed half-swap matmul
    v     = x_b @ (w_pack v slice).T          [2048 seq, 1024]  (bf16),
            written straight into a persistent SBUF tile (no DRAM roundtrip)
    per head: scoresT tiles -> exp -> causal mask
              l = ones @ eP (softmax denominators; full tiles pre-summed in
              quads on the DVE so the PE does 1/4 of the ones-matmuls)
              out_hT = sum v eP
    partial = attn_out @ w_o[:, cols].T       [2048, 4096]             (f32)
Host sums the 4 TP partials per batch (row-parallel all-reduce done on host).

Self-contained: hardcodes all shapes; only needs concourse + numpy + ml_dtypes.
"""
import math
from contextlib import ExitStack

import numpy as np
import ml_dtypes

import concourse.bass as bass
import concourse.mybir as mybir
import concourse.tile as tile
from concourse import bacc
from concourse.bass_utils import run_bass_kernel_spmd

bf16 = ml_dtypes.bfloat16
f8 = ml_dtypes.float8_e4m3
FP32 = mybir.dt.float32
BF16 = mybir.dt.bfloat16
F8E4 = mybir.dt.float8e4
DR = mybir.MatmulPerfMode.DoubleRow

B, S, H = 2, 2048, 4096
NH_TOT, HD = 32, 128
NHL = 8                # heads per core
KT = H // 128          # 32 contraction tiles for the projections
VD = NHL * HD          # 1024 local v dims
SCALE = 1.0 / math.sqrt(HD)
ROPE_BASE = 10000.0
SX = 512.0             # fp8 input scale for x
SW = 512.0             # fp8 input scale for w_pack qk rows

_NC_CACHE = {}


def build_nc():
    nc = bacc.Bacc()
    x = nc.declare_dram_parameter("x", [KT, 128, S], BF16, isOutput=False)
    x8 = nc.declare_dram_parameter("x8", [KT, 128, S], F8E4, isOutput=False)
    wqk = nc.declare_dram_parameter("wqk", [16, KT, 128, 128], F8E4, isOutput=False)
    wv = nc.declare_dram_parameter("wv", [4, 16, 128, 512], BF16, isOutput=False)
    wo = nc.declare_dram_parameter("wo", [8, 8, 128, 512], BF16, isOutput=False)
    cos = nc.declare_dram_parameter("cos", [128, S], BF16, isOutput=False)
    sin = nc.declare_dram_parameter("sin", [128, S], BF16, isOutput=False)
    ssw = nc.declare_dram_parameter("ssw", [128, 128], BF16, isOutput=False)
    msk = nc.declare_dram_parameter("msk", [128, 128], BF16, isOutput=False)
    ones = nc.declare_dram_parameter("ones", [128, 128], BF16, isOutput=False)
    out = nc.declare_dram_parameter("out", [S, H], FP32, isOutput=True)

    EXP = mybir.ActivationFunctionType.Exp

    with tile.TileContext(nc) as tc, ExitStack() as g:
        glob = g.enter_context(tc.tile_pool(name="glob", bufs=1))

        qkrot = [glob.tile([128, S], BF16, tag=f"qkrot{t}", name=f"qkrot{t}")
                 for t in range(16)]
        v_all = glob.tile([128, 16, VD], BF16, tag="vall", name="vall")
        ones_sb = glob.tile([128, 128], BF16, tag="ones", name="ones_sb")
        nc.sync.dma_start(out=ones_sb[:], in_=ones[:])

        # ---------- phase 1a: qk projection (fp8 DoubleRow) + rope ---------
        if True:
            with ExitStack() as s1:
                c1 = s1.enter_context(tc.tile_pool(name="c1", bufs=1))
                xpool = s1.enter_context(tc.tile_pool(name="xp", bufs=1))
                wpool = s1.enter_context(tc.tile_pool(name="wp", bufs=3))
                evict = s1.enter_context(tc.tile_pool(name="ev", bufs=3))
                rope = s1.enter_context(tc.tile_pool(name="rope", bufs=1))
                pp = s1.enter_context(tc.tile_pool(name="pp", bufs=2, space="PSUM"))
                pswap = s1.enter_context(tc.tile_pool(name="pswap", bufs=2, space="PSUM"))

                cos_sb = c1.tile([128, S], BF16, tag="cos", name="cos_sb")
                sin_sb = c1.tile([128, S], BF16, tag="sin", name="sin_sb")
                ssw_sb = c1.tile([128, 128], BF16, tag="ssw", name="ssw_sb")
                nc.sync.dma_start(out=cos_sb[:], in_=cos[:])
                nc.sync.dma_start(out=sin_sb[:], in_=sin[:])
                nc.sync.dma_start(out=ssw_sb[:], in_=ssw[:])

                for hf in range(2):
                    # two separate 512-col tiles: the next half's load of one
                    # slab only waits on THAT slab's readers (whole-tile WAR
                    # otherwise).  x loads ride the Scalar HWDGE queue in
                    # 4-ktile chunks so they never head-of-line-block the wqp
                    # panel stream (Sync queue) and the first matmul starts
                    # after ~256KB, not 4MB.
                    xh_lo = xpool.tile([128, KT, 512], F8E4, tag="xhlo", name=f"xhlo{hf}")
                    xh_hi = xpool.tile([128, KT, 512], F8E4, tag="xhhi", name=f"xhhi{hf}")
                    for slab, xq in ((xh_lo, 0), (xh_hi, 1)):
                        for kh in range(8):
                            nc.scalar.dma_start(
                                out=slab[:, kh * 4:(kh + 1) * 4, :],
                                in_=x8[kh * 4:(kh + 1) * 4,
                                       :, hf * 1024 + xq * 512: hf * 1024 + (xq + 1) * 512]
                                .rearrange("k p s -> p k s"),
                            )

                    # The swap-matmul + rope for block n are emitted after
                    # block n+1's projection matmuls so the PE never waits on
                    # the ACT psum eviction.  (A DMA-based half-swap was tried
                    # and regressed: its 4MB/half of SBUF-SBUF traffic delays
                    # the next half's x8 chunks on whichever HWDGE queue it
                    # rides.)
                    pending = None

                    def flush_rope(pend, hf=hf):
                        qkraw, mt_p, sb_p = pend
                        pss = pswap.tile([128, 512], FP32, tag="pss",
                                         name=f"pss{hf}{mt_p}{sb_p}")
                        nc.tensor.matmul(pss[:], lhsT=ssw_sb[:], rhs=qkraw[:],
                                         start=True, stop=True)
                        t1 = rope.tile([128, 512], BF16, tag="t1", name=f"t1_{hf}{mt_p}{sb_p}")
                        t2 = rope.tile([128, 512], BF16, tag="t2", name=f"t2_{hf}{mt_p}{sb_p}")
                        nc.vector.tensor_mul(t1[:], qkraw[:], cos_sb[:, sb_p:sb_p + 512])
                        nc.vector.tensor_mul(t2[:], pss[:], sin_sb[:, sb_p:sb_p + 512])
                        nc.vector.tensor_add(qkrot[mt_p][:, sb_p:sb_p + 512], t1[:], t2[:])

                    # nb2-outer so each 512-col slab of xh has its last reader
                    # at the end of one sub-phase: the next half's x DMA for
                    # that slab overlaps the other slab's compute
                    for nb2 in range(2):
                        for mt in range(16):
                            wqp = wpool.tile([128, KT, 128], F8E4, tag="wqp",
                                             name=f"wqp{hf}{nb2}{mt}")
                            # two DMAs so the panel loads stay ahead of the PE
                            nc.sync.dma_start(out=wqp[:, 0:16, :],
                                              in_=wqk[mt, 0:16].rearrange("k p c -> p k c"))
                            nc.sync.dma_start(out=wqp[:, 16:32, :],
                                              in_=wqk[mt, 16:32].rearrange("k p c -> p k c"))
                            sb = hf * 1024 + nb2 * 512
                            pqk = pp.tile([128, 512], FP32, tag="pqk", name=f"pqk{hf}{mt}{nb2}")
                            xslab = xh_lo if nb2 == 0 else xh_hi
                            for k2 in range(0, KT, 2):
                                nc.tensor.matmul(
                                    pqk[:],
                                    lhsT=wqp[:, k2:k2 + 2, :],
                                    rhs=xslab[:, k2:k2 + 2, :],
                                    start=(k2 == 0),
                                    stop=(k2 == KT - 2),
                                    perf_mode=DR,
                                )
                            qkraw = evict.tile([128, 512], BF16, tag="qkraw",
                                               name=f"qkraw{hf}{mt}{nb2}")
                            nc.scalar.copy(qkraw[:], pqk[:])
                            if pending is not None:
                                flush_rope(pending)
                            pending = (qkraw, mt, sb)
                    if pending is not None:
                        flush_rope(pending)
                        pending = None

            # ------ phase 1b: v projection (bf16), straight into SBUF ------
            with ExitStack() as s1b:
                xpoolv = s1b.enter_context(tc.tile_pool(name="xpv", bufs=1))
                wpool = s1b.enter_context(tc.tile_pool(name="wpb", bufs=2))
                pp = s1b.enter_context(tc.tile_pool(name="ppb", bufs=2, space="PSUM"))

                def load_vx(hf, xq):
                    # 8 chunks alternating between the two HWDGE queues: the
                    # slab fills at ~2x one queue's rate at phase entry
                    slab = xpoolv.tile([128, KT, 512], BF16, tag=f"vxh{xq}",
                                       name=f"vxh{hf}{xq}")
                    for kh in range(8):
                        eng = nc.scalar if kh % 2 == 0 else nc.sync
                        eng.dma_start(
                            out=slab[:, kh * 4:(kh + 1) * 4, :],
                            in_=x[kh * 4:(kh + 1) * 4,
                                  :, hf * 1024 + xq * 512: hf * 1024 + (xq + 1) * 512]
                            .rearrange("k p s -> p k s"),
                        )
                    return slab

                for hf in range(2):
                    xh_lo = load_vx(hf, 0)
                    xh_hi = load_vx(hf, 1)

                    def xh_chunk(k, c0, w, xh_lo=xh_lo, xh_hi=xh_hi):
                        # columns [c0, c0+w) of this half's x, no slab cross
                        slab = xh_lo if c0 < 512 else xh_hi
                        cc = c0 % 512
                        assert cc + w <= 512
                        return slab[:, k, cc:cc + w]

                    # v projection: v[seq, vdim] natural layout; N=512 panels
                    # split into two half-K chunks so a bufs=2 pool
                    # double-buffers 1MB DMAs
                    for nb in range(2):
                        panels = []
                        for kh in range(2):
                            wvp = wpool.tile([128, 16, 512], BF16, tag="wvp",
                                             name=f"wvp{hf}{nb}{kh}")
                            nc.sync.dma_start(out=wvp[:, 0:8, :],
                                              in_=wv[2 * nb + kh, 0:8].rearrange("k p c -> p k c"))
                            nc.sync.dma_start(out=wvp[:, 8:16, :],
                                              in_=wv[2 * nb + kh, 8:16].rearrange("k p c -> p k c"))
                            panels.append(wvp)
                        for mt in range(8):
                            pv = pp.tile([128, 512], FP32, tag="pv", name=f"pv{hf}{nb}{mt}")
                            for k in range(KT):
                                nc.tensor.matmul(
                                    pv[:],
                                    lhsT=xh_chunk(k, mt * 128, 128),
                                    rhs=panels[k // 16][:, k % 16, :],
                                    start=(k == 0),
                                    stop=(k == KT - 1),
                                )
                            st = hf * 8 + mt
                            # DVE eviction keeps the ACT queue free so the next
                            # half's x-slab DMA triggers fire immediately
                            nc.vector.tensor_copy(
                                v_all[:, st, nb * 512:(nb + 1) * 512], pv[:])

        # ---------- phases 2+3: attention, then output projection ----------
        with ExitStack() as s2:
            c2 = s2.enter_context(tc.tile_pool(name="c2", bufs=1))
            apool = s2.enter_context(tc.tile_pool(name="ap", bufs=1))
            eppool = s2.enter_context(tc.tile_pool(name="ep", bufs=6))
            sqpool = s2.enter_context(tc.tile_pool(name="sq", bufs=3))
            s2p = s2.enter_context(ExitStack())
            att_ps = s2p.enter_context(tc.tile_pool(name="attps", bufs=3, space="PSUM"))
            av_ps = s2p.enter_context(tc.tile_pool(name="avps", bufs=2, space="PSUM"))
            l_ps = s2p.enter_context(tc.tile_pool(name="lps", bufs=1, space="PSUM"))
            bc_ps = s2p.enter_context(tc.tile_pool(name="bcps", bufs=1, space="PSUM"))

            msk_sb = c2.tile([128, 128], BF16, tag="msk", name="msk_sb")
            nc.sync.dma_start(out=msk_sb[:], in_=msk[:])

            attnT = [apool.tile([128, S], BF16, tag=f"attnT{t}", name=f"attnT{t}")
                     for t in range(8)]

            # one l bank pair reused across all j; garbage rows only ever feed
            # unused reciprocal lanes
            lA = l_ps.tile([128, 512], FP32, tag="lA", name="lA")
            lB = l_ps.tile([128, 512], FP32, tag="lB", name="lB")
            nc.vector.memset(lA[:], 1.0)
            nc.vector.memset(lB[:], 1.0)

            # linv tiles allocated up front: their 128-col reciprocal chunks
            # are emitted ONE PER (j,h) GROUP during the next j's groups, so
            # the 0.85us iterative-divide ops never pile up on the DVE queue
            # ahead of the mask/quad ops the PE pipeline depends on.
            linvs = [
                (apool.tile([128, 512], BF16, tag=f"linvA{j}", name=f"linvA{j}"),
                 apool.tile([128, 512], BF16, tag=f"linvB{j}", name=f"linvB{j}"))
                for j in range(4)
            ]
            lsd = {}

            def emit_recip(jsrc, idx):
                bank = 0 if idx < 4 else 1
                cc = (idx % 4) * 128
                src = lsd[(jsrc, bank)]
                dst = linvs[jsrc][bank]
                with nc.allow_low_precision(reason="bf16 1/l: +1e-3 rel err, single-pass bcast matmul"):
                    nc.vector.reciprocal(dst[:, cc:cc + 128], src[:, cc:cc + 128])

            for j in range(4):
                ni = 4 * j + 4
                for h in range(8):
                    lbank = lA if h < 4 else lB
                    hp = (h % 4) * 32
                    pav = av_ps.tile([128, 512], FP32, tag="pav", name=f"pav{j}{h}")

                    eps = {}

                    def c_lo(i, j=j):
                        r = i - 4 * j
                        return 128 * r if r > 0 else 0

                    def emit_score(i, j=j, h=h):
                        # diagonal tiles only need columns >= 128*(i-4j); the
                        # strip [c0, c0+128) gets the triangular mask
                        c0 = c_lo(i)
                        psc = att_ps.tile([128, 512], FP32, tag="psc",
                                          name=f"psc{j}{h}{i}")
                        nc.tensor.matmul(
                            psc[:, c0:512],
                            lhsT=qkrot[8 + h][:, i * 128:(i + 1) * 128],
                            rhs=qkrot[h][:, j * 512 + c0:(j + 1) * 512],
                            start=True, stop=True,
                        )
                        ep = eppool.tile([128, 512], BF16, tag="ep",
                                         name=f"ep{j}{h}{i}")
                        nc.scalar.activation(ep[:, c0:512], psc[:, c0:512],
                                             EXP, scale=SCALE)
                        if i - 4 * j >= 0:
                            nc.vector.tensor_mul(ep[:, c0:c0 + 128],
                                                 ep[:, c0:c0 + 128], msk_sb[:])
                        eps[i] = ep

                    # l reduction plan: full tiles (i < 4j) are pre-summed in
                    # quads on the DVE (one ones-matmul per 4 tiles); the 4
                    # ragged diagonal tiles go straight to the PE.  The quad
                    # l-matmuls are deferred to group end so the PE (in-order)
                    # never waits on the DVE adds.
                    quad = []   # full-width ep tiles awaiting quad reduction
                    qqs = []    # reduced quad tiles awaiting their l-matmul

                    def flush_quad(j=j, h=h):
                        assert len(quad) == 4
                        nq = len(qqs)
                        q0 = sqpool.tile([128, 512], BF16, tag="q0",
                                         name=f"q0_{j}{h}{nq}")
                        q1 = sqpool.tile([128, 512], BF16, tag="q1",
                                         name=f"q1_{j}{h}{nq}")
                        qq = sqpool.tile([128, 512], BF16, tag="qq",
                                         name=f"qq_{j}{h}{nq}")
                        nc.vector.tensor_add(q0[:], quad[0][:], quad[1][:])
                        nc.vector.tensor_add(q1[:], quad[2][:], quad[3][:])
                        nc.vector.tensor_add(qq[:], q0[:], q1[:])
                        qqs.append(qq)
                        quad.clear()

                    # software-pipeline: scores run 3 tiles ahead of l/av so the
                    # exp+mask latency never stalls the PE
                    LOOKAHEAD = 3
                    for i in range(ni + LOOKAHEAD):
                        if i < ni:
                            emit_score(i)
                        ii = i - LOOKAHEAD
                        if ii < 0:
                            continue
                        ep = eps.pop(ii)
                        c0 = c_lo(ii)
                        if ii < 4 * j:
                            quad.append(ep)
                            if len(quad) == 4:
                                flush_quad()
                        else:
                            # ragged diagonal tile: direct l-matmul.  These run
                            # first in the group (quads are deferred), so the
                            # first one (full-width, c0=0) carries start=True.
                            nc.tensor.matmul(
                                lbank[hp:hp + 1, c0:512],
                                lhsT=ones_sb[:, 0:1],
                                rhs=ep[:, c0:512],
                                start=(ii == 4 * j),
                                stop=(j == 0 and ii == ni - 1),
                                tile_position=(0, hp),
                            )
                        nc.tensor.matmul(
                            pav[:, c0:512],
                            lhsT=v_all[:, ii, h * 128:(h + 1) * 128],
                            rhs=ep[:, c0:512],
                            start=(ii == 0), stop=(ii == ni - 1),
                        )
                    assert not quad
                    for qi, qq in enumerate(qqs):
                        nc.tensor.matmul(
                            lbank[hp:hp + 1, :],
                            lhsT=ones_sb[:, 0:1],
                            rhs=qq[:],
                            start=False, stop=(qi == len(qqs) - 1),
                            tile_position=(0, hp),
                        )
                    # DVE copy: keeps the ScalarE exp-only during attention (no
                    # activation-table thrash between Copy and Exp)
                    nc.vector.tensor_copy(attnT[h][:, j * 512:(j + 1) * 512], pav[:])
                    # evict each l bank right after its LAST writer (lA: h==3,
                    # lB: h==7) so the next j's first l-matmul never waits on a
                    # queued DVE copy (WAR on the bank)
                    if h == 3:
                        lsA = apool.tile([128, 512], FP32, tag=f"lsA{j}", name=f"lsA{j}")
                        nc.vector.tensor_copy(lsA[:], lA[:])
                        lsd[(j, 0)] = lsA
                    elif h == 7:
                        lsB = apool.tile([128, 512], FP32, tag=f"lsB{j}", name=f"lsB{j}")
                        nc.vector.tensor_copy(lsB[:], lB[:])
                        lsd[(j, 1)] = lsB
                    # one reciprocal chunk per group, spread so they never
                    # head-of-line-block the DVE
                    if j > 0:
                        emit_recip(j - 1, h)
                    if j == 3 and h >= 4:
                        emit_recip(3, h - 4)
            for idx in range(4, 8):
                emit_recip(3, idx)

            # normalize attn_outT by 1/l (broadcast 1/l across partitions)
            for j in range(4):
                for h in range(8):
                    linv = linvs[j][0] if h < 4 else linvs[j][1]
                    hp = (h % 4) * 32
                    bc = bc_ps.tile([128, 512], FP32, tag="bc", name=f"bc{j}{h}")
                    nc.tensor.matmul(
                        bc[:],
                        lhsT=ones_sb[hp:hp + 1, :],
                        rhs=linv[hp:hp + 1, :],
                        start=True, stop=True,
                        tile_position=(hp, 0),
                    )
                    nc.vector.tensor_mul(
                        attnT[h][:, j * 512:(j + 1) * 512],
                        attnT[h][:, j * 512:(j + 1) * 512],
                        bc[:],
                    )

            # ---------- phase 3: output projection --------------------------
            s2p.close()
            with ExitStack() as s3:
                wpool3 = s3.enter_context(tc.tile_pool(name="wp3", bufs=2))
                ev3 = s3.enter_context(tc.tile_pool(name="ev3", bufs=3))
                po_ps = s3.enter_context(tc.tile_pool(name="pops", bufs=2, space="PSUM"))
                for nb in range(8):
                    # wop rides the Scalar HWDGE queue so it is never stuck
                    # behind the 4MB/nb of output-tile writes on the Sync queue
                    wop = wpool3.tile([128, 8, 512], BF16, tag="wop", name=f"wop{nb}")
                    nc.scalar.dma_start(out=wop[:, 0:4, :],
                                        in_=wo[nb, 0:4].rearrange("k p c -> p k c"))
                    nc.scalar.dma_start(out=wop[:, 4:8, :],
                                        in_=wo[nb, 4:8].rearrange("k p c -> p k c"))
                    for mt in range(16):
                        po = po_ps.tile([128, 512], FP32, tag="po", name=f"po{nb}{mt}")
                        for k in range(8):
                            nc.tensor.matmul(
                                po[:],
                                lhsT=attnT[k][:, mt * 128:(mt + 1) * 128],
                                rhs=wop[:, k, :],
                                start=(k == 0), stop=(k == 7),
                            )
                        osb = ev3.tile([128, 512], FP32, tag="osb", name=f"osb{nb}{mt}")
                        nc.scalar.copy(osb[:], po[:])
                        nc.sync.dma_start(
                            out=out[mt * 128:(mt + 1) * 128, nb * 512:(nb + 1) * 512],
                            in_=osb[:],
                        )

    nc.finalize()
    return nc


def _rope_tables(pos_row):
    """cos/sin tables [128, S]: row p uses inv_freq[p % 64]; the 1/(SX*SW)
    fp8 descale for q,k is folded in."""
    inv = 1.0 / (ROPE_BASE ** (np.arange(0, HD, 2, dtype=np.float32) / HD))  # [64]
    inv128 = np.concatenate([inv, inv]).astype(np.float32)                   # [128]
    ang = inv128[:, None] * pos_row[None, :].astype(np.float32)              # [128, S]
    ds = 1.0 / (SX * SW)
    return (np.cos(ang) * ds).astype(bf16), (np.sin(ang) * ds).astype(bf16)


def _consts():
    ssw = np.zeros((128, 128), np.float32)
    for f in range(64):
        ssw[64 + f, f] = -1.0   # out[f]    = -in[64+f]
        ssw[f, 64 + f] = 1.0    # out[64+f] = +in[f]
    # triangular tile mask: msk[p, c] = 1 iff c >= p
    msk = np.triu(np.ones((128, 128), np.float32))
    ones = np.ones((128, 128), np.float32)
    return ssw.astype(bf16), msk.astype(bf16), ones.astype(bf16)


def prep_in_maps(hidden_states, w_pack, w_o, positions):
    hidden_states = np.asarray(hidden_states, dtype=np.float32)
    w_pack = np.asarray(w_pack, dtype=np.float32)
    w_o = np.asarray(w_o, dtype=np.float32)
    positions = np.asarray(positions)

    ssw, msk, ones = _consts()
    in_maps = []
    for c in range(8):
        b, g = divmod(c, 4)
        xT = np.ascontiguousarray(hidden_states[b].T)                  # [H, S]
        x_np = xT.astype(bf16).reshape(KT, 128, S)
        x8_np = np.clip(xT * SX, -240, 240).astype(f8).reshape(KT, 128, S)
        qbase = g * 1024
        kbase = H + g * 1024
        vbase = 2 * H + g * 1024
        wqk_np = np.empty((16, KT, 128, 128), f8)
        for mt in range(16):
            base = qbase + 128 * mt if mt < 8 else kbase + 128 * (mt - 8)
            blk = w_pack[base:base + 128, :]                      # [128, H]
            wqk_np[mt] = np.clip(blk.T * SW, -240, 240).astype(f8).reshape(KT, 128, 128)
        wv_np = np.empty((4, 16, 128, 512), bf16)
        for nb in range(2):
            blk = w_pack[vbase + 512 * nb: vbase + 512 * (nb + 1), :]  # [512, H]
            arr = blk.T.astype(bf16).reshape(2, 16, 128, 512)          # [kh, kk, p, c]
            wv_np[2 * nb] = arr[0]
            wv_np[2 * nb + 1] = arr[1]
        woT = np.ascontiguousarray(w_o[:, g * 1024:(g + 1) * 1024].T)  # [1024, H]
        wo_np = np.ascontiguousarray(
            woT.reshape(8, 128, 8, 512).transpose(2, 0, 1, 3)
        ).astype(bf16)
        cos_np, sin_np = _rope_tables(positions[b])
        in_maps.append({
            "x": x_np, "x8": x8_np, "wqk": wqk_np, "wv": wv_np, "wo": wo_np,
            "cos": cos_np, "sin": sin_np,
            "ssw": ssw, "msk": msk, "ones": ones,
        })
    return in_maps


def kernel(hidden_states, w_pack, w_o, positions, _run_kwargs=None):
    if "nc" not in _NC_CACHE:
        _NC_CACHE["nc"] = build_nc()
    nc = _NC_CACHE["nc"]
    in_maps = prep_in_maps(hidden_states, w_pack, w_o, positions)
    res = run_bass_kernel_spmd(nc, in_maps, core_ids=list(range(8)),
                               **(_run_kwargs or {}))
    _NC_CACHE["last_result"] = res
    out = np.zeros((B, S, H), np.float32)
    for c in range(8):
        b = c // 4
        out[b] += res.results[c]["out"]
    return out


# revision 33
# speedup vs baseline: 1.0556x; 1.0455x over previous
"""Trainium2 Bass kernel for Baichuan attention (B=2, S=2048, H=4096, 32 heads).

Sharding: 8 cores = 2 (batch) x 4 (head groups of 8 heads), tensor-parallel
mirror of ColumnParallelLinear/RowParallelLinear. Each core computes, for its
batch b and head group g:
    qkT   = (w_pack q,k slice) @ x_b.T        [2048 qkdims, 2048 seq]
            in fp8e4 DoubleRow (x,w scaled by 512 each; descale folded into
            the cos/sin rope tables), rope via a signed half-swap matmul
    v     = x_b @ (w_pack v slice).T          [2048 seq, 1024]  (bf16),
            written straight into a persistent SBUF tile (no DRAM roundtrip)
    per head: scoresT tiles -> exp -> causal mask
              l = ones @ eP (softmax denominators; full tiles pre-summed in
              quads on the DVE so the PE does 1/4 of the ones-matmuls)
              out_hT = sum v eP
    partial = attn_out @ w_o[:, cols].T       [2048, 4096]             (f32)
Host sums the 4 TP partials per batch (row-parallel all-reduce done on host).

Self-contained: hardcodes all shapes; only needs concourse + numpy + ml_dtypes.
"""
import math
from contextlib import ExitStack

import numpy as np
import ml_dtypes

import concourse.bass as bass
import concourse.mybir as mybir
import concourse.tile as tile
from concourse import bacc
from concourse.bass_utils import run_bass_kernel_spmd

bf16 = ml_dtypes.bfloat16
f8 = ml_dtypes.float8_e4m3
FP32 = mybir.dt.float32
BF16 = mybir.dt.bfloat16
F8E4 = mybir.dt.float8e4
DR = mybir.MatmulPerfMode.DoubleRow

B, S, H = 2, 2048, 4096
NH_TOT, HD = 32, 128
NHL = 8                # heads per core
KT = H // 128          # 32 contraction tiles for the projections
VD = NHL * HD          # 1024 local v dims
SCALE = 1.0 / math.sqrt(HD)
ROPE_BASE = 10000.0
SX = 512.0             # fp8 input scale for x
SW = 512.0             # fp8 input scale for w_pack qk rows

_NC_CACHE = {}


def build_nc():
    nc = bacc.Bacc()
    x = nc.declare_dram_parameter("x", [KT, 128, S], BF16, isOutput=False)
    x8 = nc.declare_dram_parameter("x8", [KT, 128, S], F8E4, isOutput=False)
    wqk = nc.declare_dram_parameter("wqk", [16, KT, 128, 128], F8E4, isOutput=False)
    wv = nc.declare_dram_parameter("wv", [4, 16, 128, 512], BF16, isOutput=False)
    wo = nc.declare_dram_parameter("wo", [8, 8, 128, 512], BF16, isOutput=False)
    cos = nc.declare_dram_parameter("cos", [128, S], BF16, isOutput=False)
    sin = nc.declare_dram_parameter("sin", [128, S], BF16, isOutput=False)
    ssw = nc.declare_dram_parameter("ssw", [128, 128], BF16, isOutput=False)
    msk = nc.declare_dram_parameter("msk", [128, 128], BF16, isOutput=False)
    ones = nc.declare_dram_parameter("ones", [128, 128], BF16, isOutput=False)
    out = nc.declare_dram_parameter("out", [S, H], FP32, isOutput=True)

    EXP = mybir.ActivationFunctionType.Exp

    with tile.TileContext(nc) as tc, ExitStack() as g:
        glob = g.enter_context(tc.tile_pool(name="glob", bufs=1))

        qkrot = [glob.tile([128, S], BF16, tag=f"qkrot{t}", name=f"qkrot{t}")
                 for t in range(16)]
        v_all = glob.tile([128, 16, VD], BF16, tag="vall", name="vall")
        ones_sb = glob.tile([128, 128], BF16, tag="ones", name="ones_sb")
        nc.sync.dma_start(out=ones_sb[:], in_=ones[:])

        # ---------- phase 1a: qk projection (fp8 DoubleRow) + rope ---------
        if True:
            with ExitStack() as s1:
                c1 = s1.enter_context(tc.tile_pool(name="c1", bufs=1))
                xpool = s1.enter_context(tc.tile_pool(name="xp", bufs=1))
                wpool = s1.enter_context(tc.tile_pool(name="wp", bufs=3))
                evict = s1.enter_context(tc.tile_pool(name="ev", bufs=3))
                rope = s1.enter_context(tc.tile_pool(name="rope", bufs=1))
                pp = s1.enter_context(tc.tile_pool(name="pp", bufs=2, space="PSUM"))
                pswap = s1.enter_context(tc.tile_pool(name="pswap", bufs=2, space="PSUM"))

                cos_sb = c1.tile([128, S], BF16, tag="cos", name="cos_sb")
                sin_sb = c1.tile([128, S], BF16, tag="sin", name="sin_sb")
                ssw_sb = c1.tile([128, 128], BF16, tag="ssw", name="ssw_sb")
                nc.sync.dma_start(out=cos_sb[:], in_=cos[:])
                nc.sync.dma_start(out=sin_sb[:], in_=sin[:])
                nc.sync.dma_start(out=ssw_sb[:], in_=ssw[:])

                for hf in range(2):
                    # two separate 512-col tiles: the next half's load of one
                    # slab only waits on THAT slab's readers (whole-tile WAR
                    # otherwise).  x loads ride the Scalar HWDGE queue in
                    # 4-ktile chunks so they never head-of-line-block the wqp
                    # panel stream (Sync queue) and the first matmul starts
                    # after ~256KB, not 4MB.
                    xh_lo = xpool.tile([128, KT, 512], F8E4, tag="xhlo", name=f"xhlo{hf}")
                    xh_hi = xpool.tile([128, KT, 512], F8E4, tag="xhhi", name=f"xhhi{hf}")
                    for slab, xq in ((xh_lo, 0), (xh_hi, 1)):
                        for kh in range(8):
                            nc.scalar.dma_start(
                                out=slab[:, kh * 4:(kh + 1) * 4, :],
                                in_=x8[kh * 4:(kh + 1) * 4,
                                       :, hf * 1024 + xq * 512: hf * 1024 + (xq + 1) * 512]
                                .rearrange("k p s -> p k s"),
                            )

                    # The swap-matmul + rope for block n are emitted after
                    # block n+1's projection matmuls so the PE never waits on
                    # the ACT psum eviction.  (A DMA-based half-swap was tried
                    # and regressed: its 4MB/half of SBUF-SBUF traffic delays
                    # the next half's x8 chunks on whichever HWDGE queue it
                    # rides.)
                    pending = None

                    def flush_rope(pend, hf=hf):
                        qkraw, mt_p, sb_p = pend
                        pss = pswap.tile([128, 512], FP32, tag="pss",
                                         name=f"pss{hf}{mt_p}{sb_p}")
                        nc.tensor.matmul(pss[:], lhsT=ssw_sb[:], rhs=qkraw[:],
                                         start=True, stop=True)
                        t1 = rope.tile([128, 512], BF16, tag="t1", name=f"t1_{hf}{mt_p}{sb_p}")
                        t2 = rope.tile([128, 512], BF16, tag="t2", name=f"t2_{hf}{mt_p}{sb_p}")
                        nc.vector.tensor_mul(t1[:], qkraw[:], cos_sb[:, sb_p:sb_p + 512])
                        nc.vector.tensor_mul(t2[:], pss[:], sin_sb[:, sb_p:sb_p + 512])
                        nc.vector.tensor_add(qkrot[mt_p][:, sb_p:sb_p + 512], t1[:], t2[:])

                    # nb2-outer so each 512-col slab of xh has its last reader
                    # at the end of one sub-phase: the next half's x DMA for
                    # that slab overlaps the other slab's compute
                    for nb2 in range(2):
                        for mt in range(16):
                            wqp = wpool.tile([128, KT, 128], F8E4, tag="wqp",
                                             name=f"wqp{hf}{nb2}{mt}")
                            # two DMAs so the panel loads stay ahead of the PE
                            nc.sync.dma_start(out=wqp[:, 0:16, :],
                                              in_=wqk[mt, 0:16].rearrange("k p c -> p k c"))
                            nc.sync.dma_start(out=wqp[:, 16:32, :],
                                              in_=wqk[mt, 16:32].rearrange("k p c -> p k c"))
                            sb = hf * 1024 + nb2 * 512
                            pqk = pp.tile([128, 512], FP32, tag="pqk", name=f"pqk{hf}{mt}{nb2}")
                            xslab = xh_lo if nb2 == 0 else xh_hi
                            for k2 in range(0, KT, 2):
                                nc.tensor.matmul(
                                    pqk[:],
                                    lhsT=wqp[:, k2:k2 + 2, :],
                                    rhs=xslab[:, k2:k2 + 2, :],
                                    start=(k2 == 0),
                                    stop=(k2 == KT - 2),
                                    perf_mode=DR,
                                )
                            qkraw = evict.tile([128, 512], BF16, tag="qkraw",
                                               name=f"qkraw{hf}{mt}{nb2}")
                            nc.scalar.copy(qkraw[:], pqk[:])
                            if pending is not None:
                                flush_rope(pending)
                            pending = (qkraw, mt, sb)
                    if pending is not None:
                        flush_rope(pending)
                        pending = None

            # ------ phase 1b: v projection (bf16), straight into SBUF ------
            with ExitStack() as s1b:
                xpoolv = s1b.enter_context(tc.tile_pool(name="xpv", bufs=1))
                wpool = s1b.enter_context(tc.tile_pool(name="wpb", bufs=2))
                pp = s1b.enter_context(tc.tile_pool(name="ppb", bufs=2, space="PSUM"))

                def load_vx(hf, xq):
                    # Scalar HWDGE queue only: riding the Sync queue too was
                    # tried and regressed (x chunks head-of-line-block the wvp
                    # panel stream and vice versa)
                    slab = xpoolv.tile([128, KT, 512], BF16, tag=f"vxh{xq}",
                                       name=f"vxh{hf}{xq}")
                    for kh in range(4):
                        nc.scalar.dma_start(
                            out=slab[:, kh * 8:(kh + 1) * 8, :],
                            in_=x[kh * 8:(kh + 1) * 8,
                                  :, hf * 1024 + xq * 512: hf * 1024 + (xq + 1) * 512]
                            .rearrange("k p s -> p k s"),
                        )
                    return slab

                for hf in range(2):
                    xh_lo = load_vx(hf, 0)
                    xh_hi = load_vx(hf, 1)

                    def xh_chunk(k, c0, w, xh_lo=xh_lo, xh_hi=xh_hi):
                        # columns [c0, c0+w) of this half's x, no slab cross
                        slab = xh_lo if c0 < 512 else xh_hi
                        cc = c0 % 512
                        assert cc + w <= 512
                        return slab[:, k, cc:cc + w]

                    # v projection: v[seq, vdim] natural layout; N=512 panels
                    # split into two half-K chunks so a bufs=2 pool
                    # double-buffers 1MB DMAs
                    for nb in range(2):
                        panels = []
                        for kh in range(2):
                            wvp = wpool.tile([128, 16, 512], BF16, tag="wvp",
                                             name=f"wvp{hf}{nb}{kh}")
                            nc.sync.dma_start(out=wvp[:, 0:8, :],
                                              in_=wv[2 * nb + kh, 0:8].rearrange("k p c -> p k c"))
                            nc.sync.dma_start(out=wvp[:, 8:16, :],
                                              in_=wv[2 * nb + kh, 8:16].rearrange("k p c -> p k c"))
                            panels.append(wvp)
                        for mt in range(8):
                            pv = pp.tile([128, 512], FP32, tag="pv", name=f"pv{hf}{nb}{mt}")
                            for k in range(KT):
                                nc.tensor.matmul(
                                    pv[:],
                                    lhsT=xh_chunk(k, mt * 128, 128),
                                    rhs=panels[k // 16][:, k % 16, :],
                                    start=(k == 0),
                                    stop=(k == KT - 1),
                                )
                            st = hf * 8 + mt
                            # DVE eviction keeps the ACT queue free so the next
                            # half's x-slab DMA triggers fire immediately
                            nc.vector.tensor_copy(
                                v_all[:, st, nb * 512:(nb + 1) * 512], pv[:])

        # ---------- phases 2+3: attention, then output projection ----------
        with ExitStack() as s2:
            c2 = s2.enter_context(tc.tile_pool(name="c2", bufs=1))
            apool = s2.enter_context(tc.tile_pool(name="ap", bufs=1))
            eppool = s2.enter_context(tc.tile_pool(name="ep", bufs=8))
            sqpool = s2.enter_context(tc.tile_pool(name="sq", bufs=3))
            s2p = s2.enter_context(ExitStack())
            att_ps = s2p.enter_context(tc.tile_pool(name="attps", bufs=4, space="PSUM"))
            av_ps = s2p.enter_context(tc.tile_pool(name="avps", bufs=2, space="PSUM"))
            l_ps = s2p.enter_context(tc.tile_pool(name="lps", bufs=1, space="PSUM"))

            msk_sb = c2.tile([128, 128], BF16, tag="msk", name="msk_sb")
            nc.sync.dma_start(out=msk_sb[:], in_=msk[:])

            attnT = [apool.tile([128, S], BF16, tag=f"attnT{t}", name=f"attnT{t}")
                     for t in range(8)]

            # one l bank pair reused across all j; garbage rows only ever feed
            # unused reciprocal lanes
            lA = l_ps.tile([128, 512], FP32, tag="lA", name="lA")
            lB = l_ps.tile([128, 512], FP32, tag="lB", name="lB")
            nc.vector.memset(lA[:], 1.0)
            nc.vector.memset(lB[:], 1.0)

            # linv tiles allocated up front: their 128-col reciprocal chunks
            # are emitted ONE PER (j,h) GROUP during the next j's groups, so
            # the 0.85us iterative-divide ops never pile up on the DVE queue
            # ahead of the mask/quad ops the PE pipeline depends on.
            linvs = [
                (apool.tile([128, 512], BF16, tag=f"linvA{j}", name=f"linvA{j}"),
                 apool.tile([128, 512], BF16, tag=f"linvB{j}", name=f"linvB{j}"))
                for j in range(4)
            ]
            lsd = {}

            def emit_recip(jsrc, idx):
                bank = 0 if idx < 4 else 1
                cc = (idx % 4) * 128
                src = lsd[(jsrc, bank)]
                dst = linvs[jsrc][bank]
                # negative offset = LATER priority: the scheduler then runs the
                # group's mask/quad DVE ops first and slots the reciprocal into
                # DVE idle time instead of ahead of them (head-of-line stalls
                # on the PE measured otherwise)
                with tc.high_priority(offset=-60), \
                     nc.allow_low_precision(reason="bf16 1/l: +1e-3 rel err, single-pass bcast matmul"):
                    nc.vector.reciprocal(dst[:, cc:cc + 128], src[:, cc:cc + 128])

            for j in range(4):
                ni = 4 * j + 4
                for h in range(8):
                    lbank = lA if h < 4 else lB
                    hp = (h % 4) * 32
                    pav = av_ps.tile([128, 512], FP32, tag="pav", name=f"pav{j}{h}")

                    eps = {}

                    def c_lo(i, j=j):
                        r = i - 4 * j
                        return 128 * r if r > 0 else 0

                    def emit_score(i, j=j, h=h):
                        # diagonal tiles only need columns >= 128*(i-4j); the
                        # strip [c0, c0+128) gets the triangular mask
                        c0 = c_lo(i)
                        psc = att_ps.tile([128, 512], FP32, tag="psc",
                                          name=f"psc{j}{h}{i}")
                        nc.tensor.matmul(
                            psc[:, c0:512],
                            lhsT=qkrot[8 + h][:, i * 128:(i + 1) * 128],
                            rhs=qkrot[h][:, j * 512 + c0:(j + 1) * 512],
                            start=True, stop=True,
                        )
                        ep = eppool.tile([128, 512], BF16, tag="ep",
                                         name=f"ep{j}{h}{i}")
                        nc.scalar.activation(ep[:, c0:512], psc[:, c0:512],
                                             EXP, scale=SCALE)
                        if i - 4 * j >= 0:
                            nc.vector.tensor_mul(ep[:, c0:c0 + 128],
                                                 ep[:, c0:c0 + 128], msk_sb[:])
                        eps[i] = ep

                    # l reduction plan: full tiles (i < 4j) are pre-summed in
                    # quads on the DVE (one ones-matmul per 4 tiles); the 4
                    # ragged diagonal tiles go straight to the PE.  The quad
                    # l-matmuls are deferred to group end so the PE (in-order)
                    # never waits on the DVE adds.
                    quad = []   # full-width ep tiles awaiting quad reduction
                    qqs = []    # reduced quad tiles awaiting their l-matmul

                    def flush_quad(j=j, h=h):
                        assert len(quad) == 4
                        nq = len(qqs)
                        q0 = sqpool.tile([128, 512], BF16, tag="q0",
                                         name=f"q0_{j}{h}{nq}")
                        q1 = sqpool.tile([128, 512], BF16, tag="q1",
                                         name=f"q1_{j}{h}{nq}")
                        qq = sqpool.tile([128, 512], BF16, tag="qq",
                                         name=f"qq_{j}{h}{nq}")
                        nc.vector.tensor_add(q0[:], quad[0][:], quad[1][:])
                        nc.vector.tensor_add(q1[:], quad[2][:], quad[3][:])
                        nc.vector.tensor_add(qq[:], q0[:], q1[:])
                        qqs.append(qq)
                        quad.clear()

                    # software-pipeline: scores run 4 tiles ahead of l/av so the
                    # exp+mask latency never stalls the PE
                    LOOKAHEAD = 4
                    for i in range(ni + LOOKAHEAD):
                        if i < ni:
                            emit_score(i)
                        ii = i - LOOKAHEAD
                        if ii < 0:
                            continue
                        ep = eps.pop(ii)
                        c0 = c_lo(ii)
                        if ii < 4 * j:
                            quad.append(ep)
                            if len(quad) == 4:
                                flush_quad()
                        else:
                            # ragged diagonal tile: direct l-matmul.  These run
                            # first in the group (quads are deferred), so the
                            # first one (full-width, c0=0) carries start=True.
                            nc.tensor.matmul(
                                lbank[hp:hp + 1, c0:512],
                                lhsT=ones_sb[:, 0:1],
                                rhs=ep[:, c0:512],
                                start=(ii == 4 * j),
                                stop=(j == 0 and ii == ni - 1),
                                tile_position=(0, hp),
                            )
                        nc.tensor.matmul(
                            pav[:, c0:512],
                            lhsT=v_all[:, ii, h * 128:(h + 1) * 128],
                            rhs=ep[:, c0:512],
                            start=(ii == 0), stop=(ii == ni - 1),
                        )
                    assert not quad
                    for qi, qq in enumerate(qqs):
                        nc.tensor.matmul(
                            lbank[hp:hp + 1, :],
                            lhsT=ones_sb[:, 0:1],
                            rhs=qq[:],
                            start=False, stop=(qi == len(qqs) - 1),
                            tile_position=(0, hp),
                        )
                    # DVE copy: keeps the ScalarE exp-only during attention (no
                    # activation-table thrash between Copy and Exp)
                    nc.vector.tensor_copy(attnT[h][:, j * 512:(j + 1) * 512], pav[:])
                    # evict each l bank right after its LAST writer (lA: h==3,
                    # lB: h==7) so the next j's first l-matmul never waits on a
                    # queued DVE copy (WAR on the bank)
                    if h == 3:
                        lsA = apool.tile([128, 512], FP32, tag=f"lsA{j}", name=f"lsA{j}")
                        nc.vector.tensor_copy(lsA[:], lA[:])
                        lsd[(j, 0)] = lsA
                    elif h == 7:
                        lsB = apool.tile([128, 512], FP32, tag=f"lsB{j}", name=f"lsB{j}")
                        nc.vector.tensor_copy(lsB[:], lB[:])
                        lsd[(j, 1)] = lsB
                    # one reciprocal chunk per group, spread so they never
                    # head-of-line-block the DVE
                    if j > 0:
                        emit_recip(j - 1, h)
                    if j == 3 and h >= 4:
                        emit_recip(3, h - 4)
            for idx in range(4, 8):
                emit_recip(3, idx)

            # normalize attn_outT by 1/l (broadcast 1/l across partitions);
            # bc's PSUM bank comes from a fresh scope so the main loop can run
            # a 4-deep score ring within the 8-bank budget
            s2p.close()
            with ExitStack() as s2n:
                bc_ps = s2n.enter_context(tc.tile_pool(name="bcps", bufs=2, space="PSUM"))
                for j in range(4):
                    for h in range(8):
                        linv = linvs[j][0] if h < 4 else linvs[j][1]
                        hp = (h % 4) * 32
                        bc = bc_ps.tile([128, 512], FP32, tag="bc", name=f"bc{j}{h}")
                        nc.tensor.matmul(
                            bc[:],
                            lhsT=ones_sb[hp:hp + 1, :],
                            rhs=linv[hp:hp + 1, :],
                            start=True, stop=True,
                            tile_position=(hp, 0),
                        )
                        nc.vector.tensor_mul(
                            attnT[h][:, j * 512:(j + 1) * 512],
                            attnT[h][:, j * 512:(j + 1) * 512],
                            bc[:],
                        )

            # ---------- phase 3: output projection --------------------------
            with ExitStack() as s3:
                wpool3 = s3.enter_context(tc.tile_pool(name="wp3", bufs=2))
                ev3 = s3.enter_context(tc.tile_pool(name="ev3", bufs=3))
                po_ps = s3.enter_context(tc.tile_pool(name="pops", bufs=2, space="PSUM"))
                for nb in range(8):
                    # wop rides the Scalar HWDGE queue so it is never stuck
                    # behind the 4MB/nb of output-tile writes on the Sync queue
                    wop = wpool3.tile([128, 8, 512], BF16, tag="wop", name=f"wop{nb}")
                    nc.scalar.dma_start(out=wop[:, 0:4, :],
                                        in_=wo[nb, 0:4].rearrange("k p c -> p k c"))
                    nc.scalar.dma_start(out=wop[:, 4:8, :],
                                        in_=wo[nb, 4:8].rearrange("k p c -> p k c"))
                    for mt in range(16):
                        po = po_ps.tile([128, 512], FP32, tag="po", name=f"po{nb}{mt}")
                        for k in range(8):
                            nc.tensor.matmul(
                                po[:],
                                lhsT=attnT[k][:, mt * 128:(mt + 1) * 128],
                                rhs=wop[:, k, :],
                                start=(k == 0), stop=(k == 7),
                            )
                        osb = ev3.tile([128, 512], FP32, tag="osb", name=f"osb{nb}{mt}")
                        nc.scalar.copy(osb[:], po[:])
                        nc.sync.dma_start(
                            out=out[mt * 128:(mt + 1) * 128, nb * 512:(nb + 1) * 512],
                            in_=osb[:],
                        )

    nc.finalize()
    return nc


def _rope_tables(pos_row):
    """cos/sin tables [128, S]: row p uses inv_freq[p % 64]; the 1/(SX*SW)
    fp8 descale for q,k is folded in."""
    inv = 1.0 / (ROPE_BASE ** (np.arange(0, HD, 2, dtype=np.float32) / HD))  # [64]
    inv128 = np.concatenate([inv, inv]).astype(np.float32)                   # [128]
    ang = inv128[:, None] * pos_row[None, :].astype(np.float32)              # [128, S]
    ds = 1.0 / (SX * SW)
    return (np.cos(ang) * ds).astype(bf16), (np.sin(ang) * ds).astype(bf16)


def _consts():
    ssw = np.zeros((128, 128), np.float32)
    for f in range(64):
        ssw[64 + f, f] = -1.0   # out[f]    = -in[64+f]
        ssw[f, 64 + f] = 1.0    # out[64+f] = +in[f]
    # triangular tile mask: msk[p, c] = 1 iff c >= p
    msk = np.triu(np.ones((128, 128), np.float32))
    ones = np.ones((128, 128), np.float32)
    return ssw.astype(bf16), msk.astype(bf16), ones.astype(bf16)


def prep_in_maps(hidden_states, w_pack, w_o, positions):
    hidden_states = np.asarray(hidden_states, dtype=np.float32)
    w_pack = np.asarray(w_pack, dtype=np.float32)
    w_o = np.asarray(w_o, dtype=np.float32)
    positions = np.asarray(positions)

    ssw, msk, ones = _consts()
    in_maps = []
    for c in range(8):
        b, g = divmod(c, 4)
        xT = np.ascontiguousarray(hidden_states[b].T)                  # [H, S]
        x_np = xT.astype(bf16).reshape(KT, 128, S)
        x8_np = np.clip(xT * SX, -240, 240).astype(f8).reshape(KT, 128, S)
        qbase = g * 1024
        kbase = H + g * 1024
        vbase = 2 * H + g * 1024
        wqk_np = np.empty((16, KT, 128, 128), f8)
        for mt in range(16):
            base = qbase + 128 * mt if mt < 8 else kbase + 128 * (mt - 8)
            blk = w_pack[base:base + 128, :]                      # [128, H]
            wqk_np[mt] = np.clip(blk.T * SW, -240, 240).astype(f8).reshape(KT, 128, 128)
        wv_np = np.empty((4, 16, 128, 512), bf16)
        for nb in range(2):
            blk = w_pack[vbase + 512 * nb: vbase + 512 * (nb + 1), :]  # [512, H]
            arr = blk.T.astype(bf16).reshape(2, 16, 128, 512)          # [kh, kk, p, c]
            wv_np[2 * nb] = arr[0]
            wv_np[2 * nb + 1] = arr[1]
        woT = np.ascontiguousarray(w_o[:, g * 1024:(g + 1) * 1024].T)  # [1024, H]
        wo_np = np.ascontiguousarray(
            woT.reshape(8, 128, 8, 512).transpose(2, 0, 1, 3)
        ).astype(bf16)
        cos_np, sin_np = _rope_tables(positions[b])
        in_maps.append({
            "x": x_np, "x8": x8_np, "wqk": wqk_np, "wv": wv_np, "wo": wo_np,
            "cos": cos_np, "sin": sin_np,
            "ssw": ssw, "msk": msk, "ones": ones,
        })
    return in_maps


def kernel(hidden_states, w_pack, w_o, positions, _run_kwargs=None):
    if "nc" not in _NC_CACHE:
        _NC_CACHE["nc"] = build_nc()
    nc = _NC_CACHE["nc"]
    in_maps = prep_in_maps(hidden_states, w_pack, w_o, positions)
    res = run_bass_kernel_spmd(nc, in_maps, core_ids=list(range(8)),
                               **(_run_kwargs or {}))
    _NC_CACHE["last_result"] = res
    out = np.zeros((B, S, H), np.float32)
    for c in range(8):
        b = c // 4
        out[b] += res.results[c]["out"]
    return out
